# revision 1
# baseline (speedup 1.0000x reference)
"""AttentionPairBias Trainium2 kernel (8-core SPMD, row-sharded).

Sharding: core c owns query rows i in [128c, 128c+128) and the matching z
rows z[:, i_shard, :, :]. k/v shards are computed from each core's own rows
and AllGathered. The pair-bias path computes LayerNorm stats with bn_stats
(two j's per op via the even/odd interleave), projects raw z16 through wz on
the PE (per-j transposes), and folds LN mean/rstd in as a post-matmul
correction:
    bias_h(i,j) = rs_ij * (P_raw_h(i,j) - mu_ij * c1_h)   [+ const_h, dropped:
softmax is shift-invariant per row]. z_norm_w is folded into wz, z_norm_b
drops with the constant. No softmax max-subtraction: logits are O(1) by
construction (|logit| < ~4), exact in fp32 exp.
"""
import numpy as np

import concourse.bass as bass
import concourse.tile as tile_mod
from concourse import mybir
from concourse.tile import TileContext
from concourse.masks import make_identity
from concourse.vector_clock import ScopedClock

F32 = mybir.dt.float32
F16 = mybir.dt.float16

S = 1024          # sequence length
DS = 1024         # model dim
H = 16            # heads
HD = 64           # head dim
DZ = 128          # pair dim
NCORES = 8
SI = S // NCORES  # 128 query rows per core


# ---------------------------------------------------------------------------
# Framework patch: this walrus build accepts only ONE semaphore wait per
# instruction, but TileContext's final drain aggregates every outstanding sem
# wait onto a single SP Drain. Split the waits across a chain of Drains.
# ---------------------------------------------------------------------------
def _patched_drain_and_barrier(self, tick_clock, wait_clock):
    nc = self.nc
    drain_inst = nc.sync.drain()
    wait_clock.add_sem_waits(
        drain_inst.ins, ScopedClock({None: tick_clock.global_clock})
    )
    si = drain_inst.ins.sync_info
    if si is not None and si.on_wait is not None and len(si.on_wait) > 1:
        extra = list(si.on_wait[1:])
        del si.on_wait[1:]
        for w in extra:
            d2 = nc.sync.drain()
            si2 = d2.ins.sync_info
            if si2 is None:
                d2.ins.sync_info = mybir.SyncInfo(on_wait=[w], on_update=[])
            else:
                si2.on_wait.append(w)
    nc.all_engine_barrier()
    assert self.sems is not None
    popped = nc._tile_sem_poison_stack.pop()
    assert popped is self._sem_poison
    nc.clear_and_free_semaphores(list(self.sems.allocated().values()))
    nc.all_engine_barrier()


def _install_patches():
    tile_mod.TileContext._drain_and_barrier = _patched_drain_and_barrier


_install_patches()


def _split_multiwait(nc):
    """This walrus build accepts at most one semaphore wait per instruction;
    Tile emits more when an op depends on producers on several engines. Hoist
    all-but-one wait onto same-engine NOPs inserted just before. (HW/walrus
    only — CoreSim can't run the unregistered NOPs.)"""
    for fn in nc.m.functions:
        for bb in fn.blocks:
            out = []
            changed = False
            for inst in bb.instructions:
                si = inst.sync_info
                if si is not None and si.on_wait is not None and len(si.on_wait) > 1:
                    extra = list(si.on_wait[:-1])
                    del si.on_wait[:-1]
                    for w in extra:
                        out.append(mybir.InstNoOp(
                            name=nc.get_next_instruction_name(),
                            engine=inst.engine,
                            bass_nofuse=True,
                            sync_info=mybir.SyncInfo(on_wait=[w], on_update=[]),
                        ))
                    changed = True
                out.append(inst)
            if changed:
                bb.instructions[:] = out


def _bn_stats_noopt(nc, out, in_):
    """bn_stats with opt=False AP lowering (keeps the interleaved view)."""
    return nc.vector.add_instruction(
        mybir.InstBNStats(
            name=nc.get_next_instruction_name(),
            ins=[nc.vector.lower_ap(in_, opt=False)],
            outs=[nc.vector.lower_ap(out, opt=False)],
        )
    )


def _bcast(ap, dims, extra_offset=0):
    return bass.AP(tensor=ap.tensor, offset=ap.offset + extra_offset, ap=dims)


def build_nc(split_waits=True, interleave_stats=True):
    nc = bass.Bass("TRN2", target_bir_lowering=False, debug=False,
                   num_devices=NCORES)

    z_sh = nc.dram_tensor("z_sh", [SI, S, DZ], F32, kind="ExternalInput").ap()
    sTi16 = nc.dram_tensor("sTi16", [DS, SI], F16, kind="ExternalInput").ap()
    wqT16 = nc.dram_tensor("wqT16", [DS, DS], F16, kind="ExternalInput").ap()
    wkT16 = nc.dram_tensor("wkT16", [DS, DS], F16, kind="ExternalInput").ap()
    wvT16 = nc.dram_tensor("wvT16", [DS, DS], F16, kind="ExternalInput").ap()
    wgT16 = nc.dram_tensor("wgT16", [DS, DS], F16, kind="ExternalInput").ap()
    woT16 = nc.dram_tensor("woT16", [DS, DS], F16, kind="ExternalInput").ap()
    wz16 = nc.dram_tensor("wz16", [DZ, H], F16, kind="ExternalInput").ap()
    c1h = nc.dram_tensor("c1h", [1, H], F32, kind="ExternalInput").ap()
    bq8 = nc.dram_tensor("bq8", [DS, 1], F32, kind="ExternalInput").ap()
    out_sh = nc.dram_tensor("out_sh", [SI, DS], F32, kind="ExternalOutput").ap()

    kv_agi = nc.dram_tensor("kv_agi", [SI, 2 * DS], F16)
    kv_ago = nc.dram_tensor("kv_ago", [S, 2 * DS], F16, addr_space="Shared")

    with TileContext(nc, pool_alloc_mode="queue") as tc:
        _emit(nc, tc, z_sh, sTi16, wqT16, wkT16, wvT16, wgT16, woT16,
              wz16, c1h, bq8, out_sh, kv_agi, kv_ago, interleave_stats)
    if split_waits:
        _split_multiwait(nc)
    return nc


def _emit(nc, tc, z_sh, sTi16, wqT16, wkT16, wvT16, wgT16, woT16,
          wz16, c1h, bq8, out_sh, kv_agi, kv_ago, interleave_stats):
    from contextlib import ExitStack
    AL = mybir.AluOpType
    AF = mybir.ActivationFunctionType

    KT = 8   # 1024/128 K tiles
    G = 8    # j-group size in the z pipeline
    NG = S // G          # 128 groups
    JB = 32              # j's per P psum bank
    NB = S // JB         # 32 P banks
    RND = 256            # j's per stats-finalize round
    NR = S // RND        # 4 rounds

    ctx = ExitStack()
    with ctx:
        consts = ctx.enter_context(tc.tile_pool(name="consts", bufs=1))
        persist = ctx.enter_context(tc.tile_pool(name="persist", bufs=1))

        ident16 = consts.tile([128, 128], F16)
        make_identity(nc, ident16)
        wz_sb = consts.tile([DZ, H], F16)
        nc.sync.dma_start(out=wz_sb, in_=wz16)
        c1h_sb = consts.tile([128, H], F32)
        nc.sync.dma_start(out=c1h_sb, in_=_bcast(c1h, [[0, 128], [1, H]]))
        bq_sb = consts.tile([128, KT], F32)
        nc.sync.dma_start(out=bq_sb, in_=bq8.rearrange("(m p) o -> p (m o)", p=128))
        eps_sb = consts.tile([128, 1], F32)
        nc.vector.memset(eps_sb, 1e-5)

        # persistent SBUF tensors
        kT_sb = persist.tile([128, KT, S], F16)     # [d-part, d-tile, j]
        v_sb = persist.tile([128, KT, DS], F16)     # [j-part, j-tile, d]
        qT_sb = persist.tile([128, KT, SI], F16)    # [d-part, d-tile, i]
        g16 = persist.tile([128, DS], F16)          # [i, d]
        st_sb = persist.tile([128, S // 2, 6], F32)  # bn_stats (j-pair, 6)
        rs = persist.tile([128, S], F32)            # 1/sqrt(var+eps)
        murs = persist.tile([128, S], F32)          # mu*rs
        sums = persist.tile([128, H], F32)
        inv = persist.tile([128, H], F32)
        og16 = persist.tile([128, DS], F16)
        ogT_sb = persist.tile([128, KT, SI], F16)
        out_sb = persist.tile([128, DS], F32)

        # ---------------- Phase A: projections + kv AllGather ----------------
        with (
            tc.tile_pool(name="wpool", bufs=1) as wpool,
            tc.tile_pool(name="apsum", bufs=2, space="PSUM") as apsum,
        ):
            sTi_sb = wpool.tile([128, KT, SI], F16)
            nc.sync.dma_start(
                out=sTi_sb, in_=sTi16.rearrange("(m p) n -> p m n", p=128))
            wq_sb = wpool.tile([128, KT, DS], F16)
            nc.sync.dma_start(
                out=wq_sb, in_=wqT16.rearrange("(m p) n -> p m n", p=128))
            wk_sb = wpool.tile([128, KT, DS], F16)
            nc.sync.dma_start(
                out=wk_sb, in_=wkT16.rearrange("(m p) n -> p m n", p=128))
            wv_sb = wpool.tile([128, KT, DS], F16)
            nc.sync.dma_start(
                out=wv_sb, in_=wvT16.rearrange("(m p) n -> p m n", p=128))
            wg_sb = wpool.tile([128, KT, DS], F16)
            nc.sync.dma_start(
                out=wg_sb, in_=wgT16.rearrange("(m p) n -> p m n", p=128))

            # k/v shards for own rows: [128 j, 1024 d], then AllGather
            kv_sh = wpool.tile([128, 2, DS], F16)
            for which, w_sb in ((0, wk_sb), (1, wv_sb)):
                for n in range(2):
                    kvp = apsum.tile([128, 512], F32, tag="kvp")
                    for k in range(KT):
                        nc.tensor.matmul(kvp, sTi_sb[:, k, :],
                                         w_sb[:, k, 512 * n:512 * (n + 1)],
                                         start=(k == 0), stop=(k == KT - 1))
                    nc.any.tensor_copy(kv_sh[:, which, 512 * n:512 * (n + 1)], kvp)
            nc.sync.dma_start(
                out=kv_agi.ap().rearrange("p (w n) -> p w n", w=2), in_=kv_sh)

            # qT[d, i] += bq  (wq, bq pre-scaled by 1/8 on host)
            for m in range(KT):
                qp = apsum.tile([128, SI], F32, tag="qp")
                for k in range(KT):
                    nc.tensor.matmul(qp, wq_sb[:, k, 128 * m:128 * (m + 1)],
                                     sTi_sb[:, k, :],
                                     start=(k == 0), stop=(k == KT - 1))
                nc.vector.tensor_scalar(
                    out=qT_sb[:, m, :], in0=qp, scalar1=bq_sb[:, m:m + 1],
                    scalar2=None, op0=AL.add)

            # g = sigmoid(s_i @ wg^T)   [i, d]
            for n in range(2):
                gp = apsum.tile([128, 512], F32, tag="gp")
                for k in range(KT):
                    nc.tensor.matmul(gp, sTi_sb[:, k, :],
                                     wg_sb[:, k, 512 * n:512 * (n + 1)],
                                     start=(k == 0), stop=(k == KT - 1))
                nc.scalar.activation(g16[:, 512 * n:512 * (n + 1)], gp,
                                     AF.Sigmoid)

        # P/bias buffers live B..C; allocated after phase A frees the weights
        biasp = ctx.enter_context(tc.tile_pool(name="biasp", bufs=1))
        P16 = biasp.tile([128, S, H], F16)          # P_raw [i, j, h]
        bias32 = biasp.tile([128, S, H], F32)       # corrected bias

        # ---------------- Phase B: z pipeline ----------------
        with (
            tc.tile_pool(name="zpool", bufs=8) as zpool,
            tc.tile_pool(name="ztpool", bufs=3) as ztpool,
            tc.tile_pool(name="zpsum", bufs=3, space="PSUM") as zpsum,
            tc.tile_pool(name="ppsum", bufs=2, space="PSUM") as ppsum,
            tc.tile_pool(name="stmp", bufs=2) as stmp,
        ):
            def finalize_round(r):
                # stats finalize (per parity; bn_stats cols: even j ->
                # [count, mean, M2] = 0..2, odd j -> 3..5)
                pr = slice(RND * r // 2, RND * (r + 1) // 2)   # pair indices
                for par in range(2):
                    stm = st_sb[:, pr, 1 + 3 * par:2 + 3 * par]
                    st2 = st_sb[:, pr, 2 + 3 * par:3 + 3 * par]
                    # strided output views over j (stride 2)
                    ro = _bcast(rs, [list(rs.ap[0]), [2, RND // 2], [0, 1]],
                                extra_offset=RND * r + par)
                    mo = _bcast(murs, [list(murs.ap[0]), [2, RND // 2], [0, 1]],
                                extra_offset=RND * r + par)
                    veps = stmp.tile([128, RND // 2, 1], F32, tag="veps")
                    nc.vector.tensor_scalar_mul(veps, st2, 1.0 / DZ)
                    sq = stmp.tile([128, RND // 2, 1], F32, tag="sq")
                    nc.scalar.activation(sq, veps, AF.Sqrt, bias=eps_sb)
                    nc.vector.reciprocal(ro, sq)
                    nc.vector.tensor_tensor(out=mo, in0=stm, in1=ro, op=AL.mult)

            def correct_bank(b):
                # bias = rs*P_raw - (mu*rs) x c1
                jb = slice(JB * b, JB * (b + 1))
                rs_rep = _bcast(rs, [list(rs.ap[0]), [1, JB], [0, H]],
                                extra_offset=JB * b)
                murs_rep = _bcast(murs, [list(murs.ap[0]), [1, JB], [0, H]],
                                  extra_offset=JB * b)
                c1_rep = _bcast(c1h_sb, [list(c1h_sb.ap[0]), [0, JB], [1, H]])
                t1 = stmp.tile([128, JB, H], F32, tag="t1")
                nc.vector.tensor_tensor(out=t1, in0=P16[:, jb, :], in1=rs_rep,
                                        op=AL.mult)
                t2 = stmp.tile([128, JB, H], F32, tag="t2")
                nc.gpsimd.tensor_tensor(out=t2, in0=murs_rep, in1=c1_rep,
                                        op=AL.mult)
                nc.gpsimd.tensor_tensor(out=bias32[:, jb, :], in0=t1, in1=t2,
                                        op=AL.subtract)

            pbank = None
            for jg in range(NG):
                j0 = jg * G
                z16 = zpool.tile([128, G, DZ], F16, tag="z16")
                nc.gpsimd.dma_start(out=z16, in_=z_sh[:, j0:j0 + G, :])

                # LayerNorm stats. Interleaved: one bn_stats per j-PAIR with
                # an even/odd interleave view [z-step 1 x 128, j-step 128 x 2]
                # -> even stats = first j, odd stats = second j.
                for t in range(G // 2):
                    iv = _bcast(z16, [list(z16.ap[0]), [1, DZ], [DZ, 2]],
                                extra_offset=2 * t * DZ)
                    _bn_stats_noopt(nc, st_sb[:, j0 // 2 + t, :], iv)

                # transpose each [128i, 128z] -> [128z, 128i] (f16, one bank)
                ztb = zpsum.tile([128, G, 128], F16, tag="ztb")
                for t in range(G):
                    nc.tensor.transpose(ztb[:, t, :], z16[:, t, :], ident16)
                zt_sb = ztpool.tile([128, G, 128], F16, tag="zt")
                nc.any.tensor_copy(zt_sb, ztb)

                # P_raw[i, h] per j, packed 32 j per psum bank
                if jg % 4 == 0:
                    pbank = ppsum.tile([128, JB, H], F32, tag="pbank")
                for t in range(G):
                    jj = (jg % 4) * G + t
                    nc.tensor.matmul(pbank[:, jj, :], zt_sb[:, t, :], wz_sb,
                                     start=True, stop=True)
                if jg % 4 == 3:
                    b = jg // 4
                    nc.any.tensor_copy(
                        P16[:, JB * b:JB * (b + 1), :], pbank)

                if jg == 12:
                    # collective on the gpsimd queue, emitted mid-loop so the
                    # z-load pipeline is already buffered ahead of the stall
                    nc.gpsimd.collective_compute(
                        "AllGather", AL.bypass, ins=[kv_agi.ap()],
                        outs=[kv_ago.ap()],
                        replica_groups=[list(range(NCORES))])
                if jg == 28:
                    # unpack the gathered k/v; build kT via PE transposes
                    kv_view = kv_ago.ap().rearrange(
                        "(t p) (w n) -> p t w n", p=128, w=2)
                    nc.sync.dma_start(out=v_sb, in_=kv_view[:, :, 1, :])
                    for m in range(KT):
                        knm = stmp.tile([128, KT, 128], F16, tag="knm")
                        nc.sync.dma_start(
                            out=knm, in_=kv_view[:, :, 0, 128 * m:128 * (m + 1)])
                        ktp = zpsum.tile([128, KT, 128], F16, tag="ktp")
                        for t in range(KT):
                            nc.tensor.transpose(ktp[:, t, :], knm[:, t, :],
                                                ident16)
                        nc.any.tensor_copy(
                            kT_sb[:, m, :].rearrange("p (t n) -> p t n", n=128),
                            ktp)
                # pipeline the finalize + corrections: after the last group
                # of round r, finalize its stats and correct its 8 banks.
                if (jg + 1) % (RND // G) == 0:
                    r = (jg + 1) // (RND // G) - 1
                    finalize_round(r)
                    for b in range(r * (RND // JB), (r + 1) * (RND // JB)):
                        correct_bank(b)

        # ---------------- Phase C: attention ----------------
        with (
            tc.tile_pool(name="scps", bufs=2, space="PSUM") as scps,
            tc.tile_pool(name="atps", bufs=2, space="PSUM") as atps,
            tc.tile_pool(name="ops", bufs=1, space="PSUM") as ops,
            tc.tile_pool(name="attn", bufs=2) as attnp,
        ):
            ob = ops.tile([128, 2, 8, HD], F32)
            for h in range(H):
                m, p0 = h // 2, 64 * (h % 2)
                scp = scps.tile([128, 2, 512], F32, tag="scp")
                for n in range(2):
                    nc.tensor.matmul(scp[:, n, :],
                                     qT_sb[p0:p0 + 64, m, :],
                                     kT_sb[p0:p0 + 64, m, 512 * n:512 * (n + 1)],
                                     start=True, stop=True)
                sc_sb = attnp.tile([128, S], F32, tag="sc")
                nc.vector.tensor_tensor(
                    out=sc_sb, in0=scp.rearrange("p a b -> p (a b)"),
                    in1=bias32[:, :, h], op=AL.add)
                attn16 = attnp.tile([128, S], F16, tag="at")
                nc.scalar.activation(attn16, sc_sb, AF.Exp)
                nc.vector.tensor_reduce(
                    out=sums[:, h:h + 1], in_=attn16, axis=mybir.AxisListType.X,
                    op=AL.add)
                atb = atps.tile([128, G, 128], F16, tag="atb")
                for t in range(G):
                    nc.tensor.transpose(atb[:, t, :],
                                        attn16[:, 128 * t:128 * (t + 1)],
                                        ident16)
                attnT = attnp.tile([128, G, 128], F16, tag="atT")
                nc.any.tensor_copy(attnT, atb)
                for t in range(G):
                    nc.tensor.matmul(ob[:, h // 8, h % 8, :], attnT[:, t, :],
                                     v_sb[:, t, HD * h:HD * (h + 1)],
                                     start=(t == 0), stop=(t == G - 1))
                if h % 8 == 7:
                    hb = h // 8
                    nc.vector.reciprocal(inv[:, 8 * hb:8 * (hb + 1)],
                                         sums[:, 8 * hb:8 * (hb + 1)])
                    for hh in range(8 * hb, 8 * (hb + 1)):
                        nc.vector.scalar_tensor_tensor(
                            out=og16[:, HD * hh:HD * (hh + 1)],
                            in0=ob[:, hb, hh % 8, :],
                            scalar=inv[:, hh:hh + 1],
                            in1=g16[:, HD * hh:HD * (hh + 1)],
                            op0=AL.mult, op1=AL.mult)

        # ---------------- Phase D: output projection ----------------
        with (
            tc.tile_pool(name="wopool", bufs=1) as wopool,
            tc.tile_pool(name="dpsum", bufs=2, space="PSUM") as dpsum,
        ):
            wo_sb = wopool.tile([128, KT, DS], F16)
            nc.sync.dma_start(
                out=wo_sb, in_=woT16.rearrange("(m p) n -> p m n", p=128))
            ogb = dpsum.tile([128, G, 128], F16, tag="ogb")
            for t in range(G):
                nc.tensor.transpose(ogb[:, t, :],
                                    og16[:, 128 * t:128 * (t + 1)], ident16)
            nc.any.tensor_copy(ogT_sb.rearrange("p k n -> p (k n)"),
                               ogb.rearrange("p k n -> p (k n)"))
            for n in range(2):
                op_ = dpsum.tile([128, 512], F32, tag="op")
                for k in range(KT):
                    nc.tensor.matmul(op_, ogT_sb[:, k, :],
                                     wo_sb[:, k, 512 * n:512 * (n + 1)],
                                     start=(k == 0), stop=(k == KT - 1))
                nc.any.tensor_copy(out_sb[:, 512 * n:512 * (n + 1)], op_)
            nc.sync.dma_start(out=out_sh, in_=out_sb)


def prep_inputs(s, z, wq, bq, wk, wv, wg, z_norm_w, z_norm_b, wz, wo):
    """Host-side prep: shard + transpose/cast weights. Returns in_maps."""
    s2 = np.asarray(s)[0]                     # [S, DS]
    sT = np.ascontiguousarray(s2.T).astype(np.float16)
    wqT = np.ascontiguousarray((np.asarray(wq) / 8.0).T).astype(np.float16)
    wkT = np.ascontiguousarray(np.asarray(wk).T).astype(np.float16)
    wvT = np.ascontiguousarray(np.asarray(wv).T).astype(np.float16)
    wgT = np.ascontiguousarray(np.asarray(wg).T).astype(np.float16)
    woT = np.ascontiguousarray(np.asarray(wo).T).astype(np.float16)
    wz_f = (np.asarray(z_norm_w)[:, None] * np.asarray(wz).T)  # [DZ, H]
    wz16 = wz_f.astype(np.float16)
    # c1_h = sum_z wz16[z, h] (f16-quantized wz to match the device P matmul)
    c1h = wz16.astype(np.float32).sum(axis=0)[None, :].astype(np.float32)
    bq8 = (np.asarray(bq) / 8.0).astype(np.float32)[:, None]
    z0 = np.asarray(z)[0]                     # [S, S, DZ]

    in_maps = []
    for c in range(NCORES):
        i0 = SI * c
        in_maps.append({
            "z_sh": np.ascontiguousarray(z0[i0:i0 + SI]).astype(np.float32),
            "sTi16": np.ascontiguousarray(sT[:, i0:i0 + SI]),
            "wqT16": wqT, "wkT16": wkT, "wvT16": wvT, "wgT16": wgT,
            "woT16": woT, "wz16": wz16, "c1h": c1h, "bq8": bq8,
        })
    return in_maps


_NC_CACHE = None


def _get_nc():
    global _NC_CACHE
    if _NC_CACHE is None:
        _NC_CACHE = build_nc()
    return _NC_CACHE


def kernel(**inputs):
    from concourse.bass_utils import run_bass_kernel_spmd
    nc = _get_nc()
    in_maps = prep_inputs(**inputs)
    res = run_bass_kernel_spmd(nc, in_maps, core_ids=list(range(NCORES)))
    out = np.empty((1, S, DS), dtype=np.float32)
    for c in range(NCORES):
        out[0, SI * c:SI * (c + 1), :] = res.results[c]["out_sh"]
    return out



# revision 27
# speedup vs baseline: 1.3381x; 1.3381x over previous
"""AttentionPairBias Trainium2 kernel (8-core SPMD, row-sharded), v2.

Sharding: core c owns query rows i in [128c, 128c+128) and the matching z
rows. k/v shards are computed from each core's own rows and AllGathered.

Key structure (vs v1):
- z ships in TWO host-prepared layouts: natural f16 [i, j, z] (feeds the
  variance path) and transposed fp8e4 [z, j, i] (feeds the PE projection as
  the stationary operand) -- no device transposes, no psum->sbuf zt copies.
- LN mean-centering is folded into the wz weights on the host:
      bias_h = rs * (z . w_hat_h),  w_hat_h = znw*wz_h - (c1_h/DZ)*1
  so no mu corrections run on device. mu itself (needed only for the
  variance) comes free as a 17th ones-column of the PE projection.
- Sum z^2 comes from a Scalar-engine in-place Square over the natural tile
  plus one DVE tensor_scalar(accum_out) per j (4x DVE mode).
- rs folds the 1/64 fp8 weight prescale and 1/DZ via the Sqrt scale/bias.
- Attention is lag-pipelined in two 512-j slabs so slab 0 overlaps the
  second half of the z loop; exp produces softmax denominators via
  accum_out. No softmax max-subtraction (logits are O(1)).
"""
import numpy as np

import concourse.bass as bass
import concourse.tile as tile_mod
from concourse import mybir
from concourse.tile import TileContext
from concourse.masks import make_identity
from concourse.vector_clock import ScopedClock

F32 = mybir.dt.float32
F16 = mybir.dt.float16
F8 = mybir.dt.float8e4

S = 1024          # sequence length
DS = 1024         # model dim
H = 16            # heads
HD = 64           # head dim
DZ = 128          # pair dim
NCORES = 8
SI = S // NCORES  # 128 query rows per core

W8_SCALE = 64.0   # host prescale on w_hat so fp8 stays in normal range


# ---------------------------------------------------------------------------
# Framework patch: this walrus build accepts only ONE semaphore wait per
# instruction, but TileContext's final drain aggregates every outstanding sem
# wait onto a single SP Drain. Split the waits across a chain of Drains.
# ---------------------------------------------------------------------------
def _patched_drain_and_barrier(self, tick_clock, wait_clock):
    nc = self.nc
    drain_inst = nc.sync.drain()
    wait_clock.add_sem_waits(
        drain_inst.ins, ScopedClock({None: tick_clock.global_clock})
    )
    si = drain_inst.ins.sync_info
    if si is not None and si.on_wait is not None and len(si.on_wait) > 1:
        extra = list(si.on_wait[1:])
        del si.on_wait[1:]
        for w in extra:
            d2 = nc.sync.drain()
            si2 = d2.ins.sync_info
            if si2 is None:
                d2.ins.sync_info = mybir.SyncInfo(on_wait=[w], on_update=[])
            else:
                si2.on_wait.append(w)
    nc.all_engine_barrier()
    assert self.sems is not None
    popped = nc._tile_sem_poison_stack.pop()
    assert popped is self._sem_poison
    nc.clear_and_free_semaphores(list(self.sems.allocated().values()))
    nc.all_engine_barrier()


def _install_patches():
    tile_mod.TileContext._drain_and_barrier = _patched_drain_and_barrier


_install_patches()


def _split_multiwait(nc):
    """This walrus build accepts at most one semaphore wait per instruction;
    Tile emits more when an op depends on producers on several engines. Hoist
    all-but-one wait onto same-engine NOPs inserted just before."""
    for fn in nc.m.functions:
        for bb in fn.blocks:
            out = []
            changed = False
            for inst in bb.instructions:
                si = inst.sync_info
                if si is not None and si.on_wait is not None and len(si.on_wait) > 1:
                    extra = list(si.on_wait[:-1])
                    del si.on_wait[:-1]
                    for w in extra:
                        out.append(mybir.InstNoOp(
                            name=nc.get_next_instruction_name(),
                            engine=inst.engine,
                            bass_nofuse=True,
                            sync_info=mybir.SyncInfo(on_wait=[w], on_update=[]),
                        ))
                    changed = True
                out.append(inst)
            if changed:
                bb.instructions[:] = out


def _bcast(ap, dims, extra_offset=0):
    return bass.AP(tensor=ap.tensor, offset=ap.offset + extra_offset, ap=dims)


def build_nc(split_waits=True, debug_taps=False):
    nc = bass.Bass("TRN2", target_bir_lowering=False, debug=False,
                   num_devices=NCORES)

    zn16 = nc.dram_tensor("zn16", [SI, S, DZ], F16, kind="ExternalInput").ap()
    zt8 = nc.dram_tensor("zt8", [DZ, S, SI], F8, kind="ExternalInput").ap()
    sTi16 = nc.dram_tensor("sTi16", [DS, SI], F16, kind="ExternalInput").ap()
    wqT16 = nc.dram_tensor("wqT16", [DS, DS], F16, kind="ExternalInput").ap()
    wkT16 = nc.dram_tensor("wkT16", [DS, DS], F16, kind="ExternalInput").ap()
    wvT16 = nc.dram_tensor("wvT16", [DS, DS], F16, kind="ExternalInput").ap()
    wgT16 = nc.dram_tensor("wgT16", [DS, DS], F16, kind="ExternalInput").ap()
    woT16 = nc.dram_tensor("woT16", [DS, DS], F16, kind="ExternalInput").ap()
    w8 = nc.dram_tensor("w8", [DZ, H + 1], F8, kind="ExternalInput").ap()
    bq8 = nc.dram_tensor("bq8", [DS, 1], F32, kind="ExternalInput").ap()
    out_sh = nc.dram_tensor("out_sh", [SI, DS], F32, kind="ExternalOutput").ap()

    kv_agi = nc.dram_tensor("kv_agi", [SI, 2 * DS], F16)
    kv_ago = nc.dram_tensor("kv_ago", [S, 2 * DS], F16, addr_space="Shared")

    taps = {}
    if debug_taps:
        for name, shape, dt in [
            ("d_ssq", [128, S], F32), ("d_mus", [128, S], F16),
            ("d_rs", [128, S], F32), ("d_eb", [128, S, H], F16),
            ("d_bias", [128, S, H], F16), ("d_qT", [128, KT := 8, SI], F16),
            ("d_kT", [128, 8, S], F16), ("d_g16", [128, DS], F16),
            ("d_sums", [128, H, 2], F32), ("d_og", [128, DS], F16),
        ]:
            taps[name] = nc.dram_tensor(name, shape, dt,
                                        kind="ExternalOutput").ap()

    with TileContext(nc, pool_alloc_mode="queue") as tc:
        _emit(nc, tc, zn16, zt8, sTi16, wqT16, wkT16, wvT16, wgT16, woT16,
              w8, bq8, out_sh, kv_agi, kv_ago, taps)
    if split_waits:
        _split_multiwait(nc)
    return nc


def _emit(nc, tc, zn16, zt8, sTi16, wqT16, wkT16, wvT16, wgT16, woT16,
          w8, bq8, out_sh, kv_agi, kv_ago, taps=None):
    from contextlib import ExitStack
    AL = mybir.AluOpType
    AF = mybir.ActivationFunctionType

    KT = 8            # 1024/128 d tiles
    G = 32            # j's per z group
    NG = S // G       # 32 groups
    JB2 = 16          # j's per P-hat psum bank (16*17 = 272 fp32 <= 512)
    BLK = 128         # j's per rs-finalize block
    SLAB = 512        # j's per attention slab

    ctx = ExitStack()
    with ctx:
        consts = ctx.enter_context(tc.tile_pool(name="consts", bufs=1))
        persist = ctx.enter_context(tc.tile_pool(name="persist", bufs=1))

        ident16 = consts.tile([128, 128], F16)
        make_identity(nc, ident16)
        w8_sb = consts.tile([DZ, H + 1], F8)
        nc.sync.dma_start(out=w8_sb, in_=w8)
        bq_sb = consts.tile([128, KT], F32)
        nc.sync.dma_start(out=bq_sb, in_=bq8.rearrange("(m p) o -> p (m o)", p=128))
        eps_sb = consts.tile([128, 1], F32)
        nc.vector.memset(eps_sb, 4096.0 * 1e-5)

        # persistent SBUF tensors
        qT_sb = persist.tile([128, KT, SI], F16)    # [d-part, d-tile, i]
        g16 = persist.tile([128, DS], F16)          # [i, d]
        kT_sb = persist.tile([128, KT, S], F16)     # [hd-pair-part, m, j]
        v_sb = persist.tile([128, KT, DS], F16)     # [j-part, j-tile, d]
        ssq = persist.tile([128, S], F32)           # sum z^2 per (i, j)
        mus = persist.tile([128, S], F16)           # 64*mu per (i, j)
        rs = persist.tile([128, S], F32)            # rsqrt(var+eps)/64
        P16 = persist.tile([128, S, H], F16)        # raw z @ w_hat; then exp(bias)
        bias16 = persist.tile([128, S, H], F16)     # rs * P16
        sums2 = persist.tile([128, H, 2], F32)      # attn row sums per slab
        inv = persist.tile([128, H], F32)
        og16 = persist.tile([128, DS], F16)
        ogT_sb = persist.tile([128, KT, SI], F16)
        out_sb = persist.tile([128, DS], F32)

        zpool = ctx.enter_context(tc.tile_pool(name="zpool", bufs=2))
        ztpool = ctx.enter_context(tc.tile_pool(name="ztpool", bufs=2))
        sqpool = ctx.enter_context(tc.tile_pool(name="sqpool", bufs=2))
        scrpool = ctx.enter_context(tc.tile_pool(name="scrpool", bufs=2))
        wpool = ctx.enter_context(tc.tile_pool(name="wpool", bufs=2))
        attnp = ctx.enter_context(tc.tile_pool(name="attnp", bufs=2))
        ppsum = ctx.enter_context(
            tc.tile_pool(name="ppsum", bufs=2, space="PSUM"))
        scps = ctx.enter_context(
            tc.tile_pool(name="scps", bufs=2, space="PSUM"))
        atps = ctx.enter_context(
            tc.tile_pool(name="atps", bufs=2, space="PSUM"))
        ops = ctx.enter_context(
            tc.tile_pool(name="ops", bufs=1, space="PSUM"))

        obs = {}                                    # per-slab o accumulators
        o0_sb = persist.tile([128, DS], F32)        # slab-0 o evac / out stage

        # ---------------- Phase A: projections + kv AllGather --------------
        # Weights stream in 512-col halves (8KB) through a 2-deep ring so
        # DMA pipelines under the matmuls.
        sTi_sb = consts.tile([128, KT, SI], F16)
        nc.gpsimd.dma_start(
            out=sTi_sb, in_=sTi16.rearrange("(m p) n -> p m n", p=128))

        def load_w_half(wT16, n):
            wh = wpool.tile([128, KT, 512], F16, tag="w")
            nc.gpsimd.dma_start(
                out=wh,
                in_=wT16.rearrange("(m p) n -> p m n", p=128)[
                    :, :, 512 * n:512 * (n + 1)])
            return wh

        # k/v shards for own rows: [128 j, 1024 d], then AllGather
        kv_sh = consts.tile([128, 2, DS], F16)
        for which, wT16 in ((0, wkT16), (1, wvT16)):
            for n in range(2):
                wh = load_w_half(wT16, n)
                kvp = scps.tile([128, 512], F32, tag="scp")
                for k in range(KT):
                    nc.tensor.matmul(kvp, sTi_sb[:, k, :], wh[:, k, :],
                                     start=(k == 0), stop=(k == KT - 1))
                nc.vector.tensor_copy(kv_sh[:, which, 512 * n:512 * (n + 1)],
                                      kvp)
        nc.gpsimd.dma_start(
            out=kv_agi.ap().rearrange("p (w n) -> p w n", w=2), in_=kv_sh)
        nc.gpsimd.collective_compute(
            "AllGather", AL.bypass, ins=[kv_agi.ap()], outs=[kv_ago.ap()],
            replica_groups=[list(range(NCORES))])

        # qT[d, i] += bq  (wq, bq pre-scaled by 1/8 on host)
        for n in range(2):
            wh = load_w_half(wqT16, n)
            for mm in range(4):
                m = 4 * n + mm
                qp = scps.tile([128, 512], F32, tag="scp")
                for k in range(KT):
                    nc.tensor.matmul(qp[:, 0:SI],
                                     wh[:, k, 128 * mm:128 * (mm + 1)],
                                     sTi_sb[:, k, :],
                                     start=(k == 0), stop=(k == KT - 1))
                nc.vector.tensor_scalar(
                    out=qT_sb[:, m, :], in0=qp[:, 0:SI],
                    scalar1=bq_sb[:, m:m + 1], scalar2=None, op0=AL.add)

        # g = sigmoid(s_i @ wg^T)   [i, d]
        for n in range(2):
            wh = load_w_half(wgT16, n)
            gp = scps.tile([128, 512], F32, tag="scp")
            for k in range(KT):
                nc.tensor.matmul(gp, sTi_sb[:, k, :], wh[:, k, :],
                                 start=(k == 0), stop=(k == KT - 1))
            nc.scalar.activation(g16[:, 512 * n:512 * (n + 1)], gp, AF.Sigmoid)

        wo_half = [None, None]  # loaded late through the weight ring

        # ---------------- helpers ----------------
        def finalize_block(b):
            # rs = rsqrt(var + eps) / 64 via: u = 32*ssq - mus^2;
            # sq = sqrt(u + 4096*eps); rs = 1/sq.   (mus = 64*mu)
            jb = slice(BLK * b, BLK * (b + 1))
            t = attnp.tile([128, BLK], F32, tag="fin")
            nc.vector.tensor_tensor(out=t, in0=mus[:, jb], in1=mus[:, jb],
                                    op=AL.mult)
            u = attnp.tile([128, BLK], F32, tag="fin")
            nc.vector.scalar_tensor_tensor(
                out=u, in0=ssq[:, jb], scalar=32.0, in1=t,
                op0=AL.mult, op1=AL.subtract)
            sq = attnp.tile([128, BLK], F32, tag="fin")
            nc.scalar.activation(sq, u, AF.Sqrt, bias=eps_sb)
            nc.vector.reciprocal(rs[:, jb], sq)
            # bias16 = rs * P16 on gpsimd (2048 els/row per block), then
            # P16 block is dead -> overwrite it with exp(bias) (Scalar) so
            # attention can use exp(qk)*exp(bias).
            rs_rep = _bcast(rs, [list(rs.ap[0]), [1, BLK], [0, H]],
                            extra_offset=BLK * b)
            nc.gpsimd.tensor_tensor(
                out=bias16[:, jb, :], in0=P16[:, jb, :], in1=rs_rep,
                op=AL.mult)
            nc.scalar.activation(P16[:, jb, :], bias16[:, jb, :], AF.Exp)

        def unpack_kv():
            kv_view = kv_ago.ap().rearrange(
                "(t p) (w n) -> p t w n", p=128, w=2)
            nc.gpsimd.dma_start(out=v_sb, in_=kv_view[:, :, 1, :])
            for m in range(KT):
                knm = attnp.tile([128, KT, 128], F16, tag="knm")
                nc.gpsimd.dma_start(
                    out=knm, in_=kv_view[:, :, 0, 128 * m:128 * (m + 1)])
                ktp = atps.tile([128, KT, 128], F16, tag="atb")
                for t in range(KT):
                    nc.tensor.transpose(ktp[:, t, :], knm[:, t, :], ident16)
                nc.scalar.activation(
                    kT_sb[:, m, :].rearrange("p (t n) -> p t n", n=128),
                    ktp, AF.Copy)

        def attn_head(s, h):
            m, p0 = h // 2, 64 * (h % 2)
            js = slice(SLAB * s, SLAB * (s + 1))
            scp = scps.tile([128, SLAB], F32, tag="scp")
            nc.tensor.matmul(scp, qT_sb[p0:p0 + 64, m, :],
                             kT_sb[p0:p0 + 64, m, js],
                             start=True, stop=True)
            eq16 = attnp.tile([128, SLAB], F16, tag="eq")
            nc.scalar.activation(eq16, scp, AF.Exp)
            at16 = attnp.tile([128, SLAB], F16, tag="at")
            nc.vector.scalar_tensor_tensor(
                out=at16, in0=eq16, scalar=1.0, in1=P16[:, js, h],
                op0=AL.bypass, op1=AL.mult,
                accum_out=sums2[:, h, s:s + 1])
            atb = atps.tile([128, KT, 128], F16, tag="atb")
            for t in range(4):
                nc.tensor.transpose(atb[:, t, :],
                                    at16[:, 128 * t:128 * (t + 1)], ident16)
            attnT = attnp.tile([128, 4, 128], F16, tag="atT")
            nc.scalar.activation(attnT, atb[:, 0:4, :], AF.Copy)
            # per-slab accumulator: a start=True matmul clears the whole
            # bank's has_written bits, so cross-slab accumulation in PSUM
            # would lose earlier heads' partials. Combine slabs in SBUF.
            if s not in obs:
                obs[s] = ops.tile([128, H, HD], F32, tag="ob",
                                  name=f"ob{s}")
            for t in range(4):
                nc.tensor.matmul(obs[s][:, h, :], attnT[:, t, :],
                                 v_sb[:, 4 * s + t, HD * h:HD * (h + 1)],
                                 start=(t == 0), stop=(t == 3))

        # ---------------- Phase B: z loop (+ lag-fused slab 0) --------------
        for g in range(NG):
            j0 = G * g
            z16 = zpool.tile([128, G, DZ], F16, tag="z16")
            nc.sync.dma_start(out=z16, in_=zn16[:, j0:j0 + G, :])
            zt8g = ztpool.tile([128, G, SI], F8, tag="zt8")
            nc.sync.dma_start(out=zt8g, in_=zt8[:, j0:j0 + G, :])

            # P-hat (+ 64*mu in col 16) per j, 16 j's per psum bank
            # (bank tile is a full 512-fp32 bank; j's live at 17-col stride)
            for t in range(G // JB2):
                pbank = ppsum.tile([128, 512], F32, tag="pb")
                for jj in range(JB2):
                    nc.tensor.matmul(pbank[:, 17 * jj:17 * jj + H + 1],
                                     zt8g[:, JB2 * t + jj, :], w8_sb,
                                     start=True, stop=True)
                jsl = slice(j0 + JB2 * t, j0 + JB2 * (t + 1))
                pb_p = _bcast(pbank, [list(pbank.ap[0]), [17, JB2], [1, H]])
                nc.scalar.activation(P16[:, jsl, :], pb_p, AF.Copy)
                pb_mu = _bcast(pbank, [list(pbank.ap[0]), [17, JB2], [1, 1]],
                               extra_offset=H)
                nc.scalar.activation(
                    mus[:, jsl].rearrange("p (a b) -> p a b", b=1),
                    pb_mu, AF.Copy)

            # sum z^2 per j: square (Scalar mostly, DVE for some groups to
            # balance), then a DVE pairwise-halving tree 128->8 plus a final
            # segmented reduce. Large-FD ops dodge the per-j dispatch tax.
            sq16 = sqpool.tile([128, G, DZ], F16, tag="sq")
            if g % 4 == 3:
                nc.vector.tensor_tensor(out=sq16, in0=z16, in1=z16,
                                        op=AL.mult)
            else:
                nc.scalar.activation(sq16, z16, AF.Square)
            scr = scrpool.tile([128, G, 96], F16, tag="scr")
            nc.vector.tensor_tensor(          # 128 -> 64
                out=scr[:, :, 0:64], in0=sq16[:, :, 0:64],
                in1=sq16[:, :, 64:128], op=AL.add)
            nc.vector.tensor_tensor(          # 64 -> 32
                out=scr[:, :, 64:96], in0=scr[:, :, 0:32],
                in1=scr[:, :, 32:64], op=AL.add)
            nc.vector.tensor_tensor(          # 32 -> 16
                out=scr[:, :, 0:16], in0=scr[:, :, 64:80],
                in1=scr[:, :, 80:96], op=AL.add)
            nc.vector.tensor_tensor(          # 16 -> 8
                out=scr[:, :, 16:24], in0=scr[:, :, 0:8],
                in1=scr[:, :, 8:16], op=AL.add)
            nc.vector.tensor_reduce(          # 8 -> 1 per j
                out=ssq[:, j0:j0 + G].rearrange("p (a b) -> p a b", b=1),
                in_=scr[:, :, 16:24], axis=mybir.AxisListType.X, op=AL.add)

            if (g + 1) % (BLK // G) == 0:
                finalize_block((g + 1) // (BLK // G) - 1)
            if g == 12:
                unpack_kv()
            if g == 26 or g == 28:
                wo_half[(g - 26) // 2] = load_w_half(woT16, (g - 26) // 2)
            if g >= 16:
                attn_head(0, g - 16)

        # ---------------- Phase C: slab 1 + output ----------------
        nc.scalar.activation(o0_sb, obs[0].rearrange("p h d -> p (h d)"),
                             AF.Copy)
        for h in range(H):
            attn_head(1, h)

        nc.vector.tensor_tensor(
            out=o0_sb, in0=o0_sb,
            in1=obs[1].rearrange("p h d -> p (h d)"), op=AL.add)
        nc.vector.tensor_tensor(out=inv, in0=sums2[:, :, 0],
                                in1=sums2[:, :, 1], op=AL.add)
        nc.vector.reciprocal(inv, inv)
        for h in range(H):
            nc.vector.scalar_tensor_tensor(
                out=og16[:, HD * h:HD * (h + 1)],
                in0=o0_sb[:, HD * h:HD * (h + 1)],
                scalar=inv[:, h:h + 1], in1=g16[:, HD * h:HD * (h + 1)],
                op0=AL.mult, op1=AL.mult)

        ogb = atps.tile([128, KT, 128], F16, tag="atb")
        for t in range(KT):
            nc.tensor.transpose(ogb[:, t, :],
                                og16[:, 128 * t:128 * (t + 1)], ident16)
        nc.scalar.activation(ogT_sb.rearrange("p k n -> p (k n)"),
                             ogb.rearrange("p k n -> p (k n)"), AF.Copy)
        for n in range(2):
            op_ = scps.tile([128, 512], F32, tag="scp")
            for k in range(KT):
                nc.tensor.matmul(op_, ogT_sb[:, k, :],
                                 wo_half[n][:, k, :],
                                 start=(k == 0), stop=(k == KT - 1))
            nc.vector.tensor_copy(out_sb[:, 512 * n:512 * (n + 1)], op_)
        nc.sync.dma_start(out=out_sh, in_=out_sb)

        if taps:
            for name, tile in [
                ("d_ssq", ssq), ("d_mus", mus), ("d_rs", rs), ("d_eb", P16),
                ("d_bias", bias16), ("d_qT", qT_sb), ("d_kT", kT_sb),
                ("d_g16", g16), ("d_sums", sums2), ("d_og", og16),
            ]:
                nc.sync.dma_start(out=taps[name], in_=tile)


def prep_inputs(s, z, wq, bq, wk, wv, wg, z_norm_w, z_norm_b, wz, wo):
    """Host-side prep: shard + transpose/cast. Returns in_maps."""
    import ml_dtypes
    F8NP = mybir.dt.np(F8)

    s2 = np.asarray(s)[0]                     # [S, DS]
    sT = np.ascontiguousarray(s2.T).astype(np.float16)
    wqT = np.ascontiguousarray((np.asarray(wq) / 8.0).T).astype(np.float16)
    wkT = np.ascontiguousarray(np.asarray(wk).T).astype(np.float16)
    wvT = np.ascontiguousarray(np.asarray(wv).T).astype(np.float16)
    wgT = np.ascontiguousarray(np.asarray(wg).T).astype(np.float16)
    woT = np.ascontiguousarray(np.asarray(wo).T).astype(np.float16)
    bq8 = (np.asarray(bq) / 8.0).astype(np.float32)[:, None]

    # w_hat: fold z_norm_w and the mean-centering into wz; prescale by 64
    # so fp8e4 stays in normal range (rs on device carries the 1/64).
    w_tld = np.asarray(z_norm_w)[:, None] * np.asarray(wz).T  # [DZ, H]
    w_hat = w_tld - w_tld.mean(axis=0, keepdims=True)
    w8 = np.empty((DZ, H + 1), dtype=F8NP)
    w8[:, :H] = (w_hat * W8_SCALE).astype(F8NP)
    w8[:, H] = np.float32(0.5)  # ones column scaled: col = 64/DZ = 0.5
    # z_norm_b contributes a per-head constant -> drops under softmax.

    z0 = np.asarray(z)[0]                     # [S, S, DZ]

    in_maps = []
    for c in range(NCORES):
        i0 = SI * c
        zc8 = z0[i0:i0 + SI].astype(F8NP)     # [SI, S, DZ] quantized once
        zn16 = zc8.astype(np.float16)         # stats see the same values
        zt = np.ascontiguousarray(zc8.transpose(2, 1, 0))  # [DZ, S, SI]
        in_maps.append({
            "zn16": zn16, "zt8": zt,
            "sTi16": np.ascontiguousarray(sT[:, i0:i0 + SI]),
            "wqT16": wqT, "wkT16": wkT, "wvT16": wvT, "wgT16": wgT,
            "woT16": woT, "w8": w8, "bq8": bq8,
        })
    return in_maps


_NC_CACHE = None


def _get_nc():
    global _NC_CACHE
    if _NC_CACHE is None:
        _NC_CACHE = build_nc()
    return _NC_CACHE


def kernel(**inputs):
    from concourse.bass_utils import run_bass_kernel_spmd
    nc = _get_nc()
    in_maps = prep_inputs(**inputs)
    res = run_bass_kernel_spmd(nc, in_maps, core_ids=list(range(NCORES)))
    out = np.empty((1, S, DS), dtype=np.float32)
    for c in range(NCORES):
        out[0, SI * c:SI * (c + 1), :] = res.results[c]["out_sh"]
    return out
